# revision 19
# baseline (speedup 1.0000x reference)
"""MoE FFN (top-2 of 8 experts) on 8 Trainium2 NeuronCores, expert-parallel.

Strategy (matches the expert-parallel sharding hint):
  - Host: router (x @ router_w, softmax, top-2, renormalize) + all-to-all
    dispatch: gather each expert's tokens into a padded [D, C] block
    (transposed so the device kernel gets D on the partition axis).
  - Device (SPMD, one expert per core): y_e = gelu(x_e @ w1_e + b1_e) @ w2_e
    as two tiled matmul phases with the hidden activations resident in SBUF.
  - Host: combine: out[t] = sum_k weight[t,k] * (y_{e_k}[t] + b2[e_k]).

Self-contained: shapes hardcoded for B=2, S=2048, D=1024, H=4096, E=8, top-2.

Perf notes (measured on HW, see previous-session bench logs):
  - bf16 matmuls run at the same PE rate as fp32r but halve SBUF traffic and
    weight DMA; measured ~35us faster per kernel. rel err ~3.4e-3.
  - The kernel is PE-bound at the minimum possible cycle count
    (2 * C*D*H / 128^2 ~ 557k column-cycles); measured sustained PE rate is
    ~0.49-0.51 ns/column (~2 GHz), so ~275-290us is the practical floor.
  - LDWEIGHTS are free on HW (overlapped); wreuse/dedup/ldwopt gain nothing.
  - fp8 DoubleRow measured at exactly 2x bf16 flops/cycle, so hi/lo
    error-compensated fp8 (3 matmul terms) would be 1.5x SLOWER than bf16.
  - x-load/output DMAs are fully hidden; splitting or re-queueing them only
    adds issue overhead (~0.6us per DMA instruction on a HWDGE queue).
"""

import math
import os
from contextlib import ExitStack

import ml_dtypes
import numpy as np

import concourse.bass as bass
import concourse.tile as tile
from concourse import bacc, mybir
from concourse._compat import with_exitstack
from concourse.bass_utils import run_bass_kernel_spmd

B, S, D, H, E, TOP_K = 2, 2048, 1024, 4096, 8, 2
T = B * S
P = 128
N_CORES = 8
KC = D // P   # 8  k-chunks of the d contraction
HC = H // P   # 32 chunks of the hidden dim
DC = D // P   # 8  chunks of the output dim

_DT = {"bf16": mybir.dt.bfloat16, "fp32r": mybir.dt.float32r,
       "fp32": mybir.dt.float32}
_NPDT = {"bf16": np.dtype(ml_dtypes.bfloat16), "fp32r": np.dtype(np.float32),
         "fp32": np.dtype(np.float32)}

# matmul precision: "fp32" (exact, 1/4 PE rate), "fp32r" (TF32-like, full
# rate), "bf16" (full rate, halves weight DMA traffic + SBUF footprint).
# HW-tuned default: bf16 (measured ~35us faster than fp32r per iteration;
# rel err ~2e-3, well under the 2e-2 gate), SINGLE pass over all tokens
# (hidden block fully SBUF-resident) — weights streamed once.
DEFAULT_CFG = dict(
    mode=os.environ.get("MOE_DTYPE", "bf16"),
    chunk=512,    # max moving-operand columns per matmul
    cgran=32,     # capacity rounding granularity
    # PSUM pool bufs for phase 1 / phase 2 (0 ps1 = shared pool). 5/3 split
    # measured ~4us better than 4/4: phase 1 retires a psum chunk every
    # ~1.4us and the extra bank adds gelu-evacuation slack, while phase 2's
    # long accumulation chains only need 2-3 banks.
    ps1=5, ps2=3,
    w1b=2, w2b=3, xb=1, yb=2, hb=1, xsplit=0,
    passw=1088,   # 0 = auto from SBUF budget
    w2split=True,  # stream w2 as two half tiles (frees 8KB/partition SBUF)
    wreuse=False,  # keep stationary weights across token chunks (kc-outer)
    ldwopt=False,  # pass --enable-ldw-opt=true to walrus
    dedup=False,   # remove consecutive duplicate InstLdweights (use w/ wreuse)
)

LAST_RESULTS = None  # BassKernelResults of the most recent device run
_CACHE: dict = {}

# When True, the walrus invocation gets --enable-ldw-opt=true so redundant
# LDWEIGHTS (same stationary operand across consecutive matmuls, see
# wreuse) are removed from the PE stream.
LDWOPT_ACTIVE = False
_orig_run_command = None


def _install_ldwopt_patch():
    global _orig_run_command
    if _orig_run_command is not None:
        return
    from concourse import bass_utils as _bu
    _orig_run_command = _bu.run_command

    def patched(argv, **kwargs):
        if LDWOPT_ACTIVE:
            argv = ["--enable-ldw-opt=true" if a == "--enable-ldw-opt=false"
                    else a for a in argv]
        return _orig_run_command(argv, **kwargs)

    _bu.run_command = patched


def _chunks(width, chunk):
    # near-equal chunks of <= chunk columns (equal widths beat a ragged tail);
    # widths kept multiples of 8 (odd moving-dim fp32r matmuls fail codegen)
    k = (width + chunk - 1) // chunk
    base = min(chunk, -(-(width // k) // 8) * 8)
    out = []
    o = 0
    while o < width:
        c = min(base, width - o)
        out.append((o, c))
        o += c
    return out


def _passes(C, cfg):
    # h (the [H, pass_w] hidden activations) stays SBUF-resident per pass:
    # pass_w*HC*esize bytes/partition.
    max_w = cfg["passw"] or (1792 if cfg["mode"] == "bf16" else 768)
    n = max(1, math.ceil(C / max_w))
    return _chunks(C, math.ceil(C / n))


@with_exitstack
def _ffn_body(ctx: ExitStack, tc: tile.TileContext, yT, xgT, w1, b1, w2, C, cfg,
              reps=1):
    nc = tc.nc

    singles = ctx.enter_context(tc.tile_pool(name="singles", bufs=1))
    xpool = ctx.enter_context(tc.tile_pool(name="xg", bufs=cfg["xb"]))
    w1pool = ctx.enter_context(tc.tile_pool(name="w1", bufs=cfg["w1b"]))
    w2pool = ctx.enter_context(tc.tile_pool(name="w2", bufs=cfg["w2b"]))
    hpool = ctx.enter_context(tc.tile_pool(name="h", bufs=cfg["hb"]))
    ypool = ctx.enter_context(tc.tile_pool(name="y", bufs=cfg["yb"]))
    if cfg["ps1"]:
        ps1pool = ctx.enter_context(
            tc.tile_pool(name="ps1", bufs=cfg["ps1"], space="PSUM"))
        ps2pool = ctx.enter_context(
            tc.tile_pool(name="ps2", bufs=cfg["ps2"], space="PSUM"))
    else:  # shared single pool sized by ps2
        ps1pool = ps2pool = ctx.enter_context(
            tc.tile_pool(name="ps", bufs=cfg["ps2"], space="PSUM"))

    b1_sb = singles.tile([P, HC], mybir.dt.float32)
    nc.sync.dma_start(b1_sb, b1)

    xg_pre = None
    if cfg.get("xhoist"):
        # timing probe only: load x once outside the bench reps loop so the
        # measured per-iter excludes the x-load critical path
        in_dt = _DT[cfg["mode"]]
        xgT_r = xgT.rearrange("(kc p) c -> p kc c", p=P)
        xg_pre = singles.tile([P, KC, C], in_dt)
        nc.sync.dma_start(xg_pre, xgT_r)

    if reps == 1:
        _do_passes(tc, C, cfg, b1_sb, xgT, w1, w2, yT,
                   xpool, w1pool, w2pool, hpool, ypool, ps1pool, ps2pool,
                   xg_pre=xg_pre)
    else:
        with tc.For_i(0, reps, 1):
            _do_passes(tc, C, cfg, b1_sb, xgT, w1, w2, yT,
                       xpool, w1pool, w2pool, hpool, ypool, ps1pool, ps2pool,
                       xg_pre=xg_pre)


def _do_passes(tc, C, cfg, b1_sb, xgT, w1, w2, yT,
               xpool, w1pool, w2pool, hpool, ypool, ps1pool, ps2pool,
               xg_pre=None):
    nc = tc.nc
    in_dt = _DT[cfg["mode"]]
    h_dt = in_dt
    chunk = cfg["chunk"]

    # host pre-arranges weights into SBUF tile layout:
    #   w1: [P, HC, KC, 128]  (w1_dev[p, hc, kc, h] = w1[kc*128+p, hc*128+h])
    #   w2: [P, DC, HC, 128]  (w2_dev[p, dc, hc, d] = w2[hc*128+p, dc*128+d])
    # so each per-tile DMA reads 2-8KB contiguous per partition line.
    xgT_r = xgT.rearrange("(kc p) c -> p kc c", p=P)
    yT_r = yT.rearrange("(dc p) c -> dc p c", p=P)

    for (poff, pw) in _passes(C, cfg):
        chunks = _chunks(pw, chunk)
        if xg_pre is not None:
            xg = xg_pre[:, :, poff:poff + pw]
        else:
            xg = xpool.tile([P, KC, pw], in_dt)
            if cfg.get("xsplit"):
                # split the x DMA per column-chunk so the first matmul group
                # only waits for its own chunk, not the whole block (each
                # extra DMA costs ~0.6us issue on the sync queue)
                for (o, cw) in chunks:
                    nc.sync.dma_start(
                        xg[:, :, o:o + cw],
                        xgT_r[:, :, poff + o:poff + o + cw])
            elif cfg.get("xq") == 1:
                # software-DGE queue (gpsimd is otherwise idle): overlaps the
                # x load with the w1 stream on the SP queue at iteration start
                nc.gpsimd.dma_start(xg, xgT_r[:, :, poff:poff + pw])
            else:
                nc.sync.dma_start(xg, xgT_r[:, :, poff:poff + pw])
        h = hpool.tile([P, HC, pw], h_dt)
        tag1 = "ps1" if ps1pool is not ps2pool else "ps"
        tag2 = "ps2" if ps1pool is not ps2pool else "ps"

        # phase 1: h = gelu(x @ w1 + b1), computed as hT[hc, t] chunks
        for hc in range(HC):
            w1t = w1pool.tile([P, KC, P], in_dt)
            nc.sync.dma_start(w1t, w1[:, hc, :, :])
            if cfg["wreuse"]:
                # kc-outer: one stationary load serves every token chunk
                pss = [ps1pool.tile([P, chunk], mybir.dt.float32, tag=tag1,
                                    name=f"ps1_{hc}_{ci}")
                       for ci in range(len(chunks))]
                for kc in range(KC):
                    for ci, (o, cw) in enumerate(chunks):
                        nc.tensor.matmul(
                            pss[ci][:, :cw], w1t[:, kc, :], xg[:, kc, o:o + cw],
                            start=(kc == 0), stop=(kc == KC - 1))
                for ci, (o, cw) in enumerate(chunks):
                    nc.scalar.activation(
                        h[:, hc, o:o + cw], pss[ci][:, :cw],
                        mybir.ActivationFunctionType.Gelu,
                        bias=b1_sb[:, hc:hc + 1], scale=1.0)
            else:
                for (o, cw) in chunks:
                    ps = ps1pool.tile([P, chunk], mybir.dt.float32, tag=tag1)
                    for kc in range(KC):
                        nc.tensor.matmul(
                            ps[:, :cw], w1t[:, kc, :], xg[:, kc, o:o + cw],
                            start=(kc == 0), stop=(kc == KC - 1))
                    nc.scalar.activation(
                        h[:, hc, o:o + cw], ps[:, :cw],
                        mybir.ActivationFunctionType.Gelu,
                        bias=b1_sb[:, hc:hc + 1], scale=1.0)

        # phase 2: yT[dc, t] = (h.T @ w2) chunks
        for dc in range(DC):
            halves = None
            if cfg.get("w2split") and not cfg["wreuse"]:
                # half-size w2 tiles free 8KB/partition of SBUF (lets the
                # whole hidden activation block fit in a single pass)
                hh = HC // 2
                w2ta = w2pool.tile([P, hh, P], in_dt, tag="w2t",
                                   name=f"w2a_{poff}_{dc}")
                nc.sync.dma_start(w2ta, w2[:, dc, :hh, :])
                w2tb = w2pool.tile([P, hh, P], in_dt, tag="w2t",
                                   name=f"w2b_{poff}_{dc}")
                nc.sync.dma_start(w2tb, w2[:, dc, hh:, :])
                halves = [(w2ta, 0), (w2tb, hh)]
                w2t = None
            else:
                w2t = w2pool.tile([P, HC, P], in_dt, tag="w2t",
                                  name=f"w2t_{poff}_{dc}")
                nc.sync.dma_start(w2t, w2[:, dc, :, :])
            if cfg["wreuse"]:
                pss = [ps2pool.tile([P, chunk], mybir.dt.float32, tag=tag2,
                                    name=f"ps2_{dc}_{ci}")
                       for ci in range(len(chunks))]
                for hc in range(HC):
                    for ci, (o, cw) in enumerate(chunks):
                        nc.tensor.matmul(
                            pss[ci][:, :cw], w2t[:, hc, :], h[:, hc, o:o + cw],
                            start=(hc == 0), stop=(hc == HC - 1))
                for ci, (o, cw) in enumerate(chunks):
                    yt = ypool.tile([P, cw], mybir.dt.float32, tag="yt",
                                    name=f"yt_{dc}_{ci}")
                    nc.vector.tensor_copy(yt[:, :cw], pss[ci][:, :cw])
                    nc.sync.dma_start(
                        yT_r[dc, :, poff + o:poff + o + cw], yt[:, :cw])
            else:
                y_dt = mybir.dt.bfloat16 if cfg.get("ybf") else mybir.dt.float32
                for (o, cw) in chunks:
                    ps2 = ps2pool.tile([P, chunk], mybir.dt.float32, tag=tag2)
                    for hc in range(HC):
                        if halves is None:
                            lhsT = w2t[:, hc, :]
                        else:
                            wt, base = halves[hc // (HC // 2)]
                            lhsT = wt[:, hc - base, :]
                        nc.tensor.matmul(
                            ps2[:, :cw], lhsT, h[:, hc, o:o + cw],
                            start=(hc == 0), stop=(hc == HC - 1))
                    if cfg.get("ydma"):
                        # DMA straight from PSUM, skipping the DVE hop
                        nc.sync.dma_start(
                            yT_r[dc, :, poff + o:poff + o + cw], ps2[:, :cw])
                    else:
                        yt = ypool.tile([P, cw], y_dt, tag="yt",
                                        name=f"yt_{dc}_{o}")
                        nc.vector.tensor_copy(yt[:, :cw], ps2[:, :cw])
                        nc.sync.dma_start(
                            yT_r[dc, :, poff + o:poff + o + cw], yt[:, :cw])


def _dedupe_ldweights(nc):
    """Drop an InstLdweights that reloads exactly what the PE already holds
    (same SBUF slot/offset/pattern, no semaphores attached). Safe because the
    wreuse ordering makes duplicates strictly consecutive in PE order."""
    def sig(ap):
        return (ap.memref, ap.offset, str(ap.ap), ap.dtype)

    removed = 0
    for blk in nc.m.functions[0].blocks:
        out = []
        last_sig = None
        changed = False
        for inst in blk.instructions:
            if inst.engine == mybir.EngineType.PE:
                if isinstance(inst, mybir.InstLdweights):
                    s = sig(inst.ins[0])
                    si = inst.sync_info
                    no_sync = (si is None) or (
                        not si.on_wait and not si.on_update)
                    if s == last_sig and no_sync:
                        removed += 1
                        changed = True
                        continue
                    last_sig = s
                elif not isinstance(inst, mybir.InstMatmult):
                    last_sig = None  # drains/branches etc: invalidate
            out.append(inst)
        if changed:
            blk.instructions = out
    return removed


def _build(C, cfg, reps=1):
    key = (C, reps, tuple(sorted(cfg.items())))
    if key in _CACHE:
        return _CACHE[key]
    dt_in = _DT[cfg["mode"]]
    nc = bacc.Bacc("TRN2", target_bir_lowering=False, debug=False,
                   num_devices=N_CORES)
    xgT = nc.dram_tensor("xgT", (D, C), dt_in, kind="ExternalInput").ap()
    w1a = nc.dram_tensor("w1", (P, HC, KC, P), dt_in, kind="ExternalInput").ap()
    b1a = nc.dram_tensor("b1", (P, HC), mybir.dt.float32, kind="ExternalInput").ap()
    w2a = nc.dram_tensor("w2", (P, DC, HC, P), dt_in, kind="ExternalInput").ap()
    y_dt = mybir.dt.bfloat16 if cfg.get("ybf") else mybir.dt.float32
    yT = nc.dram_tensor("yT", (D, C), y_dt, kind="ExternalOutput").ap()
    with tile.TileContext(nc) as tc:
        _ffn_body(tc, yT, xgT, w1a, b1a, w2a, C, cfg, reps=reps)
    nc.compile()
    if cfg["dedup"]:
        _dedupe_ldweights(nc)
    _CACHE[key] = nc
    return nc


def _w1_dev(w1_e, np_in):
    # [D, H] -> [P, HC, KC, 128]: w1_dev[p, hc, kc, h] = w1[kc*128+p, hc*128+h]
    return np.ascontiguousarray(
        w1_e.reshape(KC, P, HC, P).transpose(1, 2, 0, 3)).astype(np_in)


def _w2_dev(w2_e, np_in):
    # [H, D] -> [P, DC, HC, 128]: w2_dev[p, dc, hc, d] = w2[hc*128+p, dc*128+d]
    return np.ascontiguousarray(
        w2_e.reshape(HC, P, DC, P).transpose(1, 2, 0, 3)).astype(np_in)


def _route(xf, router_w):
    """Replicates the reference router in fp32 numpy: softmax, top-2,
    renormalize. Returns per-expert token ids and combine weights."""
    logits = xf @ np.asarray(router_w, dtype=np.float32)          # [T, E]
    m = logits.max(axis=-1, keepdims=True)
    z = np.exp(logits - m)
    probs = z / z.sum(axis=-1, keepdims=True)
    idx = np.argpartition(-probs, TOP_K - 1, axis=-1)[:, :TOP_K]  # [T, 2]
    vals = np.take_along_axis(probs, idx, axis=-1)
    wn = vals / vals.sum(axis=-1, keepdims=True)

    eflat = idx.reshape(-1)
    tflat = np.repeat(np.arange(T), TOP_K)
    wflat = wn.reshape(-1).astype(np.float32)
    order = np.argsort(eflat, kind="stable")
    counts = np.bincount(eflat, minlength=E)
    starts = np.concatenate([[0], np.cumsum(counts)])
    toks, wts = [], []
    for e in range(E):
        sel = order[starts[e]:starts[e + 1]]
        toks.append(tflat[sel])
        wts.append(wflat[sel])
    return toks, wts, counts


def _capacity(counts, cfg):
    g = cfg["cgran"]
    return max(cfg["chunk"], int(math.ceil(counts.max() / g)) * g)


_WCACHE: dict = {}


def _weight_maps(w1, b1, w2, np_in):
    # weights are identical across kernel() calls — cache the device layouts
    # (keyed by content hash so changed weights can never hit stale entries)
    import hashlib
    hsh = hashlib.blake2b(digest_size=16)
    for a in (w1, b1, w2):
        hsh.update(np.ascontiguousarray(a).view(np.uint8).data)
    key = (hsh.hexdigest(), str(np_in))
    if key not in _WCACHE:
        _WCACHE.clear()
        _WCACHE[key] = [
            {"w1": _w1_dev(w1[e], np_in),
             "b1": np.ascontiguousarray(b1[e].reshape(HC, P).T),
             "w2": _w2_dev(w2[e], np_in)}
            for e in range(E)]
    return _WCACHE[key]


def _in_maps(xf, toks, counts, C, w1, b1, w2, cfg):
    np_in = _NPDT[cfg["mode"]]
    wmaps = _weight_maps(w1, b1, w2, np_in)
    maps = []
    for e in range(E):
        ce = counts[e]
        xg = np.zeros((D, C), dtype=np_in)
        xg[:, :ce] = xf[toks[e]].T.astype(np_in)
        maps.append({"xgT": xg, **wmaps[e]})
    return maps


def kernel(x, router_w, w1, b1, w2, b2):
    global LAST_RESULTS
    x = np.asarray(x, dtype=np.float32)
    w1 = np.asarray(w1, dtype=np.float32)
    b1 = np.asarray(b1, dtype=np.float32)
    w2 = np.asarray(w2, dtype=np.float32)
    b2 = np.asarray(b2, dtype=np.float32)
    cfg = dict(DEFAULT_CFG)

    xf = x.reshape(T, D)
    toks, wts, counts = _route(xf, router_w)
    C = _capacity(counts, cfg)
    if C > 1100 and cfg["mode"] != "bf16":
        # single-pass fp32 h block no longer fits in SBUF; fall back to two
        # overlapped passes (correct, slightly slower)
        cfg["passw"] = (C + 1) // 2
        cfg["hb"] = 2 if C <= 1152 else 1

    nc = _build(C, cfg)
    in_maps = _in_maps(xf, toks, counts, C, w1, b1, w2, cfg)

    global LDWOPT_ACTIVE
    LDWOPT_ACTIVE = bool(cfg["ldwopt"])
    if LDWOPT_ACTIVE:
        _install_ldwopt_patch()
    res = run_bass_kernel_spmd(nc, in_maps, core_ids=list(range(N_CORES)))
    LAST_RESULTS = res

    out = np.zeros((T, D), dtype=np.float32)
    for e in range(E):
        ce = counts[e]
        if ce == 0:
            continue
        y = res.results[e]["yT"][:, :ce].T.astype(np.float32)  # [ce, D]
        out[toks[e]] += wts[e][:, None] * (y + b2[e][None, :])
    return out.reshape(B, S, D)

